# revision 32
# baseline (speedup 1.0000x reference)
"""CoPEGate Trainium2 kernel.

Computes out[b,h,t,s] = sigmoid((Q K^T)[b,h,t,s] / sqrt(D)) * (P P^T)[t,s] / sqrt(D)
for B=2, H=12, T=2048, D=64 (fp32 in/out), distributed over 8 NeuronCores.

Sharding: the 24 (b,h) pairs are split 3-per-core (head-parallel); the
replicated positional bias is partly precomputed on host (tiles 0..5)
and partly computed on-device (tiles 6..15). No cross-device
communication. fp16 output (upcast on host) halves the HBM write
traffic; rel err ~4e-4 vs the 2e-2 gate.

Design (all numbers HW-measured on this part):

1. The pacer is the ACT (scalar) engine: sigmoid runs 1 elem/cycle/lane
   at 1.2 GHz regardless of dtype, so the 3 x T x T gate elements per
   core cost (cols + 172)/1.2 ns per op from PSUM = ~89-92 us minimum.
   Everything else is organized to keep ACT streaming back-to-back
   (measured: 33 ns of idle across the whole stream).

2. Geometry: 4 sigmoid ops of [128,1536] per 128-row tile (ops span
   head boundaries; matmuls stay 512-wide per PSUM bank). 1536 = 3
   banks x 2 ping-pong slots occupies 6 of the 8 PSUM banks and leaves
   2 banks for the on-device pos stripes -- a 2048-wide geometry would
   be ~2.3 us cheaper on ACT but leaves pos no PSUM and any extra
   tenancy in a 2-slot ring stalls ACT ~1.1 us per insertion
   (phase-stealing, measured).

3. K=128 matmuls at 2.4 GHz: stationary operands are zero-padded to
   128 contraction rows ([q;0] / [0;q], zero halves memset on idle
   engines); moving tiles pack two real operands ([k0;k1], [k2;pt])
   so no moving bandwidth is wasted. pt = P^T * D**-0.25: with
   stationary [0;pt], [0;pt].[k2;pt] = P P^T / sqrt(D) gives the
   device-side pos path sharing the h2 moving tile.

4. DMA is the second wall and varies ~10% run to run: out 24 MiB +
   inputs 2.25 MiB + shipped pos 3 MiB = ~29 MiB vs ~360-420 GB/s/core.
   Computing 10 of 16 pos tiles on-device (PE + DVE f32->f16 cast,
   ~2.4 us DVE per tile vs 31 us of DVE slack) cuts the wire enough
   that even slow-DMA runs finish the output stream with the compute
   instead of draining a multi-MiB backlog afterwards (was +9 us).

5. Pacing, ramp and tail (each measured as a distinct stall source):
   - Input DMAs are one-per-tile (HWDGE FIFO has ~0.5 us/transfer
     latency), ordered by first use, with the 16 KiB tile-0 column
     blocks of the stationaries split out ahead of the remainders.
   - pos DMAs ride the Sync HWDGE ring (SWDGE descriptor rings slow
     SDMA engines 7/15 -- a straggler engine once carried 2x packets
     and stretched the drain 10 us) and each is paced by a real data
     dependency: a 1-element gpsimd copy into the DMA's destination
     (WAW) whose source is the just-produced gate -- program order
     alone is reordered by the tile scheduler.
   - Tail: the final head's mul+DMA chain is split in halves then
     quarters so the last bytes trail the last sigmoid by ~1.5 us.

Steady state per row-tile (16 tiles):
  PE : 12 x [128(K),512] fp16 gate chunks + 4 pos chunks (~5 us)
  ACT: 4 x [128,1536] sigmoid PSUM->SBUF f16 (~1.44 us each; pacer)
  DVE: 3 x [128,2048] fp16 tensor_mul + 2 x [128,1024] pos casts
  DMA: 3 x 512 KiB output stripes (+ pos trickle)
"""

import math
import os
import sys

import numpy as np

sys.path.insert(0, "/opt/trn_rl_repo")

B, H, T, D = 2, 12, 2048, 64
N_CORES = 8
HPC = (B * H) // N_CORES  # heads per core
PT = 128  # output row-tile height (SBUF/PSUM partitions)
NT = T // PT  # row tiles
NCHUNK = 512  # matmul moving-operand free dim (one PSUM bank of fp32)
NCH = T // NCHUNK
INV_SQRT_D = 1.0 / math.sqrt(D)
WACT = 1536  # sigmoid op width: 4 ops x 3 banks per row-tile, leaving
NOPS = (HPC * T) // WACT  # 2 PSUM banks for on-device pos stripes
NSHIP = 6  # pos row-tiles shipped from host; tiles NSHIP..15 computed on-device

# pos-prefetch pacing: shipped pos tile jt's DMA is issued right after
# sigmoid-op 1 of row-tile jt-1 (a data dependency the scheduler cannot
# hoist), ~1.5 row-tiles ahead of the muls that consume it, so the pos
# stream never competes with the ramp-critical inputs and trickles at
# the rate the output stream leaves free.

_NC_CACHE = {}


def _build_nc():
    import concourse.bass as bass
    from concourse import bacc, mybir, tile

    f32 = mybir.dt.float32
    f16 = mybir.dt.float16
    Sigmoid = mybir.ActivationFunctionType.Sigmoid

    nc = bacc.Bacc("TRN2", target_bir_lowering=False)

    # Host-packed operands:
    #   QZ[h] = q_h^T [64, 2048]; the other 64 rows of each [128, T]
    #   stationary tile are memset to 0 on-device (zero rows contribute
    #   exactly 0 to the K=128 contraction, which runs the PE at 2.4GHz
    #   vs 1.2 for K=64).
    #   RHS[0] = [k0;k1], RHS[1] = [k2;pt] with pt = P^T * D**-0.25.
    #   QP = pt: with stationary [0;pt], [0;pt]x[k2;pt] = pt.pt
    #   = P P^T / sqrt(D), the on-device positional-bias path.
    #   POS[p, jt*T + c] = pos_bias[jt*128 + p, c] (fp16, pre-scaled) for
    #   the first NSHIP row-tiles -- already in SBUF layout.
    QZ = nc.dram_tensor("QZ", [HPC, D, T], f16, kind="ExternalInput")
    RHS = nc.dram_tensor("RHS", [2, 2 * D, T], f16, kind="ExternalInput")
    POS = nc.dram_tensor("POS", [PT, NSHIP * T], f16, kind="ExternalInput")
    QP = nc.dram_tensor("QP", [D, T], f16, kind="ExternalInput")
    out = nc.dram_tensor("out", [HPC, T, T], f16, kind="ExternalOutput")

    with tile.TileContext(nc) as tc:
        with tc.tile_pool(name="ins", bufs=1) as ins_pool, \
             tc.tile_pool(name="gate", bufs=3) as gate_pool, \
             tc.tile_pool(name="outs", bufs=12) as outs_pool, \
             tc.tile_pool(name="ps", bufs=2, space="PSUM") as ps_pool, \
             tc.tile_pool(name="pp", bufs=1, space="PSUM") as pp_pool:

            qz0 = ins_pool.tile([2 * D, T], f16, tag="qz0")
            qz1 = ins_pool.tile([2 * D, T], f16, tag="qz1")
            qz2 = ins_pool.tile([2 * D, T], f16, tag="qz2")
            rk = ins_pool.tile([2 * D, T], f16, tag="rk")
            rp = ins_pool.tile([2 * D, T], f16, tag="rp")
            qp = ins_pool.tile([2 * D, T], f16, tag="qp")
            pos = ins_pool.tile([PT, NT * T], f16, tag="pos")

            # Zero halves: qz0=[q0;0], qz1=[0;q1], qz2=[q2;0], qp=[0;pt].
            # GPSIMD + DVE are idle through the ramp; keep zeros off the
            # DMA wire.
            nc.gpsimd.memset(qz0[D : 2 * D, :], 0.0)
            nc.vector.memset(qz1[0:D, :], 0.0)
            nc.gpsimd.memset(qz2[D : 2 * D, :], 0.0)
            nc.vector.memset(qp[0:D, :], 0.0)

            # Ramp-critical first: tile 0 needs only the first column
            # block (16 KiB) of each stationary q operand, so those ship
            # separately ahead of the 240 KiB remainders. rp (moving,
            # 512 KiB, needed whole by the third sigmoid's matmuls) is
            # the long pole; everything after it has >=2 tiles of slack.
            # One DMA per tile otherwise: the HWDGE queue drains FIFO
            # with ~0.5 us of per-transfer latency.
            nc.sync.dma_start(out=qz0[0:D, 0:PT], in_=QZ[0][:, 0:PT])
            nc.sync.dma_start(out=rk[:, 0 : T // 2], in_=RHS[0][:, 0 : T // 2])
            nc.sync.dma_start(out=rk[:, T // 2 : T], in_=RHS[0][:, T // 2 : T])
            nc.sync.dma_start(out=qz1[D : 2 * D, 0:PT], in_=QZ[1][:, 0:PT])
            nc.sync.dma_start(out=rp, in_=RHS[1][:, :])
            nc.sync.dma_start(out=qz2[0:D, 0:PT], in_=QZ[2][:, 0:PT])
            nc.sync.dma_start(out=qz0[0:D, PT:T], in_=QZ[0][:, PT:T])
            nc.sync.dma_start(out=qz1[D : 2 * D, PT:T], in_=QZ[1][:, PT:T])
            nc.sync.dma_start(out=qz2[0:D, PT:T], in_=QZ[2][:, PT:T])
            nc.sync.dma_start(out=qp[D : 2 * D, :], in_=QP[:, :])

            def pos_fetch(jt, anchor):
                # anchor: a [1, 8] slice of data the DMA must wait for.
                # The copy's WAW overlap with the DMA dest paces the
                # fetch; the DMA rides the Sync HWDGE ring (SWDGE slows
                # SDMA engines 7/15 and straggled the output drain).
                nc.gpsimd.tensor_copy(
                    pos[0:1, jt * T : jt * T + 8], anchor
                )
                nc.sync.dma_start(
                    out=pos[:, jt * T : (jt + 1) * T],
                    in_=POS[:, jt * T : (jt + 1) * T],
                )

            pos_fetch(0, rp[0:1, 0:8])

            lhs_t = {0: qz0, 1: qz1, 2: qz2}
            rhs_t = {0: rk, 1: rk, 2: rp}

            for it in range(NT):
                last = it == NT - 1
                gate = gate_pool.tile([PT, HPC * T], f16, tag="gate")
                pslice = pos[:, it * T : (it + 1) * T]
                # 4 sigmoid ops of [128,1536] per row-tile (3 matmul
                # chunks each, spanning head boundaries freely); 1536 =
                # 3 PSUM banks x 2 buffers leaves 2 banks for the pos
                # stripes below. Muls/DMAs stay head-aligned and start
                # as soon as their head's cols are fully written (SBUF
                # read deps are region-exact).
                for k in range(NOPS):
                    ps = ps_pool.tile([PT, WACT], f32, tag="ps")
                    for c in range(WACT // NCHUNK):
                        g = k * WACT + c * NCHUNK
                        h, col = g // T, g % T
                        nc.tensor.matmul(
                            ps[:, bass.ts(c, NCHUNK)],
                            lhs_t[h][:, bass.ts(it, PT)],
                            rhs_t[h][:, col : col + NCHUNK],
                            start=True,
                            stop=True,
                        )
                    nc.scalar.activation(
                        gate[:, bass.ts(k, WACT)], ps, Sigmoid,
                        scale=INV_SQRT_D,
                    )
                    if it + 1 < NSHIP and k == 1:
                        pos_fetch(it + 1, gate[0:1, 0:8])
                for h in range(HPC):
                    hsl = slice(h * T, (h + 1) * T)
                    o = outs_pool.tile([PT, T], f16, tag="o")
                    if last and h == HPC - 1:
                        # Tail trim: the final head's mul+DMA in halves,
                        # the very last half again quartered, so the
                        # last chain is (512-mul + 128 KiB DMA).
                        for j in range(2):
                            jsl = bass.ts(j, T // 2)
                            gsl = slice(h * T + j * (T // 2),
                                        h * T + (j + 1) * (T // 2))
                            if j == 0:
                                nc.vector.tensor_mul(
                                    o[:, jsl], gate[:, gsl], pslice[:, jsl]
                                )
                                nc.sync.dma_start(
                                    out=out[h, bass.ts(it, PT), jsl],
                                    in_=o[:, jsl],
                                )
                            else:
                                for q in (2, 3):
                                    qsl = bass.ts(q, T // 4)
                                    gql = slice(h * T + q * (T // 4),
                                                h * T + (q + 1) * (T // 4))
                                    nc.vector.tensor_mul(
                                        o[:, qsl], gate[:, gql],
                                        pslice[:, qsl],
                                    )
                                    nc.sync.dma_start(
                                        out=out[h, bass.ts(it, PT), qsl],
                                        in_=o[:, qsl],
                                    )
                    else:
                        nc.vector.tensor_mul(o, gate[:, hsl], pslice)
                        nc.sync.dma_start(out=out[h, bass.ts(it, PT), :], in_=o)
                # On-device pos stripes for tiles NSHIP..15, one stripe
                # (two [128,1024] halves through the dedicated 2-bank
                # pp pool) per row-tile, ~3 tiles ahead of use: 4 PE
                # chunks + 2 DVE f32->f16 casts into the pos arena.
                # Cuts 5 MiB off the DMA wire so slow-DMA runs stay
                # ACT-bound instead of draining a backlog at the end.
                jp = it + 3
                if NSHIP <= jp < NT and it >= 2:
                    for half in range(2):
                        pp = pp_pool.tile([PT, T // 2], f32, tag="pp")
                        for c in range(2):
                            nc.tensor.matmul(
                                pp[:, bass.ts(c, NCHUNK)],
                                qp[:, bass.ts(jp, PT)],
                                rp[:, bass.ts(2 * half + c, NCHUNK)],
                                start=True,
                                stop=True,
                            )
                        nc.vector.tensor_copy(
                            pos[:, jp * T + half * (T // 2) :
                                jp * T + (half + 1) * (T // 2)],
                            pp,
                        )

    nc.finalize()
    return nc


def _get_nc():
    if "nc" not in _NC_CACHE:
        _NC_CACHE["nc"] = _build_nc()
    return _NC_CACHE["nc"]


def kernel(query, key, pos_embed_weight):
    query = np.asarray(query, dtype=np.float32)
    key = np.asarray(key, dtype=np.float32)
    pos_embed_weight = np.asarray(pos_embed_weight, dtype=np.float32)

    q = query.reshape(B * H, T, D)
    k = key.reshape(B * H, T, D)

    # Replicated positional bias, computed on host (small GEMM over the
    # replicated operand) in f32, pre-scaled, then cast once to fp16 in
    # the exact SBUF [partition, tile-major] layout the kernel reads.
    p = pos_embed_weight[:T]
    pos_bias = (p[: NSHIP * PT] @ p.T) * np.float32(INV_SQRT_D)
    posh = (
        pos_bias.astype(np.float16)
        .reshape(NSHIP, PT, T)
        .transpose(1, 0, 2)
        .reshape(PT, NSHIP * T)
    )
    posh = np.ascontiguousarray(posh)
    pt = np.ascontiguousarray(p.T * np.float32(D**-0.25)).astype(np.float16)

    in_maps = []
    for c in range(N_CORES):
        hs = [c * HPC + i for i in range(HPC)]
        qz = np.empty((HPC, D, T), dtype=np.float16)
        for i, h in enumerate(hs):
            qz[i] = q[h].T
        kT = [k[h].T.astype(np.float16) for h in hs]
        rhs = np.empty((2, 2 * D, T), dtype=np.float16)
        rhs[0, :D] = kT[0]
        rhs[0, D:] = kT[1]
        rhs[1, :D] = kT[2]
        rhs[1, D:] = pt
        in_maps.append({"QZ": qz, "RHS": rhs, "POS": posh, "QP": pt})

    from concourse.bass_utils import run_bass_kernel_spmd

    nc = _get_nc()
    try:
        res = run_bass_kernel_spmd(
            nc,
            in_maps,
            core_ids=list(range(N_CORES)),
            trace=bool(os.environ.get("KERNEL_TRACE")),
        )
    except Exception:
        # One retry for transient runtime/compile hiccups.
        res = run_bass_kernel_spmd(
            nc, in_maps, core_ids=list(range(N_CORES)), trace=False
        )
    kernel.last_results = res

    full = np.empty((B * H, T, T), dtype=np.float32)
    for c in range(N_CORES):
        full[c * HPC : (c + 1) * HPC] = res.results[c]["out"]
    return full.reshape(B, H, T, T)


kernel.last_results = None
